# revision 20
# baseline (speedup 1.0000x reference)
"""Multi-head attention (S=2048, B=2, E=1024, H=16) on 8 Trainium2 cores.

Sharding: data-parallel over batch (4 cores per batch element) x tensor-parallel
over heads (4 heads per core), Megatron-style: Wq/Wk/Wv column-sharded,
Wo row-sharded, per-core partial outputs summed (+bo) on host.

Per-core device kernel, all activations in [feature, token] layout (all
transposes are free on the host):
  qT = WqT.T @ xT -> [256, 2048]; kT likewise; v = xT.T @ WvT -> [2048, 256]
  per head pair (heads 2p, 2p+1 row-packed in the PE array for scores):
    scoresT[sk, sq] = kT_h.T @ qT_h            (K=64, tile_position packed)
    expT = exp(scoresT)                        (ACT, PSUM->SBUF, bf16 out)
    out_h[(d|den), sq] += [v_h|1].T.T @ expT   (M=65; row 64 = softmax denom)
    evict raw out+den immediately (frees PSUM; denom rows batched)
  one reciprocal over all 16 denom rows; per (pair,sq) one dual K=1 broadcast
  matmul + one multiply normalizes: outT = outU * bcast(1/den)
  finalT_partial = WoT.T @ outT -> [1024, 2048]
Softmax needs no max-subtraction here: scores ~ N(0,1) (|max| < ~7), exp is
safe in fp32/bf16 range. exp@[v|1] then divide is exactly softmax@v.
Biases: bq,bk folded into projection evictions; bv added to v (rows of
softmax sum to 1, so attn@(v+bv) = attn@v + bv); bo added on host.
Matmuls run in bf16 (PE 1 cyc/row + fast weight load); PSUM accumulation and
softmax denominators stay fp32.
"""

import numpy as np
import ml_dtypes
from contextlib import ExitStack

import concourse.bass as bass
import concourse.mybir as mybir
from concourse import bacc
import concourse.tile as tile
from concourse.bass_utils import run_bass_kernel_spmd

S, B, E, H, HD = 2048, 2, 1024, 16, 64
P = 128
NCORES = 8
CORES_PER_BATCH = 4
HEADS_PER_CORE = H // CORES_PER_BATCH      # 4
LOCAL_E = HEADS_PER_CORE * HD              # 256
VW = HD + 1                                # 65: [v_h | ones]
T = S                                      # tokens per core (one batch elem)
KT = E // P                                # 8 contraction tiles for projections
NPAIR = HEADS_PER_CORE // 2                # 2 head pairs
SQ_BLK = 512
NSQ = T // SQ_BLK                          # 4
NSK = T // P                               # 16
F32 = mybir.dt.float32
F32R = mybir.dt.float32r
BF16 = mybir.dt.bfloat16
NPBF16 = ml_dtypes.bfloat16
EXPF = mybir.ActivationFunctionType.Exp


DEBUG_DUMPS = False
BUILD_STAGE = 5  # 1=proj 2=+scores/exp 3=+PV 4=+normalize 5=full


def _build_program():
    nc = bacc.Bacc("TRN2")

    xq = nc.dram_tensor("xq", [E, T], BF16, kind="ExternalInput")
    xk = nc.dram_tensor("xk", [E, T], BF16, kind="ExternalInput")
    xv = nc.dram_tensor("xv", [E, T], BF16, kind="ExternalInput")
    wqt = nc.dram_tensor("wqt", [E, LOCAL_E], BF16, kind="ExternalInput")
    wkt = nc.dram_tensor("wkt", [E, LOCAL_E], BF16, kind="ExternalInput")
    wvt = nc.dram_tensor("wvt", [E, LOCAL_E], BF16, kind="ExternalInput")
    wot = nc.dram_tensor("wot", [LOCAL_E, E], BF16, kind="ExternalInput")
    bqh = nc.dram_tensor("bqh", [LOCAL_E], F32, kind="ExternalInput")
    bkh = nc.dram_tensor("bkh", [LOCAL_E], F32, kind="ExternalInput")
    # per head: [bv_head (64), 1.0] -> 65 columns
    bvb = nc.dram_tensor("bvb", [HEADS_PER_CORE * VW], F32R,
                         kind="ExternalInput")
    onesc = nc.dram_tensor("onesc", [P], BF16, kind="ExternalInput")
    onescr = nc.dram_tensor("onescr", [P], F32R, kind="ExternalInput")
    outp = nc.dram_tensor("outp", [E, T], F32, kind="ExternalOutput")
    dbg = {}
    if DEBUG_DUMPS:
        for nm, shp, dt_ in [("d_qT0", [P, T], BF16), ("d_kT0", [P, T], BF16),
                             ("d_vbuf", [P, NSK * HEADS_PER_CORE * VW], BF16),
                             ("d_outU0", [P, T], BF16), ("d_outT0", [P, T], BF16),
                             ("d_ex0", [P, 2 * SQ_BLK], BF16),
                             ("d_rc0", [1, SQ_BLK], F32),
                             ("d_bc0", [P, SQ_BLK], BF16),
                             ("d_bvbbc", [P, HEADS_PER_CORE * VW], F32)]:
            dbg[nm] = nc.dram_tensor(nm, shp, dt_, kind="ExternalOutput")

    with ExitStack() as ctx:
        ctx.enter_context(nc.allow_low_precision(reason="bf16 matmul pipeline"))
        tc = ctx.enter_context(tile.TileContext(nc))
        xpool = ctx.enter_context(tc.tile_pool(name="xpool", bufs=16))
        wpool = ctx.enter_context(tc.tile_pool(name="wpool", bufs=1))
        qkpool = ctx.enter_context(tc.tile_pool(name="qkpool", bufs=8))
        vpool = ctx.enter_context(tc.tile_pool(name="vpool", bufs=1))
        opool = ctx.enter_context(tc.tile_pool(name="opool", bufs=4))
        epool = ctx.enter_context(tc.tile_pool(name="epool", bufs=6))
        fpool = ctx.enter_context(tc.tile_pool(name="fpool", bufs=6))
        spool = ctx.enter_context(tc.tile_pool(name="spool", bufs=4))
        cpool = ctx.enter_context(tc.tile_pool(name="cpool", bufs=1))
        # PSUM: psc 2x[128,1024] (scores + q/k proj) = 4 banks,
        #       pso 4x[128,512] (PV out / v-proj / bcast / o-proj) = 4 banks.
        psc = ctx.enter_context(tc.tile_pool(name="psc", bufs=2, space="PSUM"))
        pso = ctx.enter_context(tc.tile_pool(name="pso", bufs=4, space="PSUM"))

        # ---- constants -------------------------------------------------
        ones_sb = cpool.tile([1, P], BF16, tag="ones")
        nc.sync.dma_start(ones_sb[:], onesc[None, :])
        onesr_sb = cpool.tile([1, P], F32R, tag="onesr")
        nc.sync.dma_start(onesr_sb[:], onescr[None, :])
        bvb_sb = cpool.tile([1, HEADS_PER_CORE * VW], F32R, tag="bvbrow")
        nc.sync.dma_start(bvb_sb[:], bvb[None, :])
        bq_sb = cpool.tile([HD, HEADS_PER_CORE], F32, tag="bq")
        nc.sync.dma_start(bq_sb[:], bqh.rearrange("(h p) -> p h", p=HD))
        bk_sb = cpool.tile([HD, HEADS_PER_CORE], F32, tag="bk")
        nc.sync.dma_start(bk_sb[:], bkh.rearrange("(h p) -> p h", p=HD))
        # broadcast [bv_h | 1] over all 128 partitions via a K=1 outer product
        bvb_ps = pso.tile([P, SQ_BLK], F32, tag="o")
        nc.tensor.matmul(bvb_ps[:, 0:HEADS_PER_CORE * VW],
                         onesr_sb[0:1, :], bvb_sb[0:1, :],
                         start=True, stop=True)
        bvb_bc = cpool.tile([P, HEADS_PER_CORE * VW], F32, tag="bvbbc")
        nc.vector.tensor_copy(bvb_bc[:], bvb_ps[:, 0:HEADS_PER_CORE * VW])

        # ---- weights ---------------------------------------------------
        wq_sb = wpool.tile([P, KT, LOCAL_E], BF16, tag="wq")
        nc.sync.dma_start(wq_sb[:], wqt.rearrange("(k p) n -> p k n", p=P))
        wk_sb = wpool.tile([P, KT, LOCAL_E], BF16, tag="wk")
        nc.sync.dma_start(wk_sb[:], wkt.rearrange("(k p) n -> p k n", p=P))
        wv_sb = wpool.tile([P, KT, LOCAL_E], BF16, tag="wv")
        nc.sync.dma_start(wv_sb[:], wvt.rearrange("(k p) n -> p k n", p=P))
        wo_sb = wpool.tile([P, LOCAL_E // P, E], BF16, tag="wo")
        nc.sync.dma_start(wo_sb[:], wot.rearrange("(k p) n -> p k n", p=P))

        # ---- persistent activations -----------------------------------
        # per-head tiles (only rows 0:HD used) so every matmul lhsT/rhs
        # sits at base partition 0 -- no PE tile_position games
        qT = [qkpool.tile([P, T], BF16, tag="qk", name=f"qT{i}")
              for i in range(HEADS_PER_CORE)]
        kT = [qkpool.tile([P, T], BF16, tag="qk", name=f"kT{i}")
              for i in range(HEADS_PER_CORE)]
        # v buffer: per sk-tile, per head: [v_h (64 cols) | ones (1 col)]
        vbuf = vpool.tile([P, NSK, HEADS_PER_CORE * VW], BF16, tag="v")
        for _tt in range(NSK):
            nc.vector.tensor_copy(
                vbuf.rearrange("p s (h c) -> p s h c", c=VW)
                [:, _tt, :, HD:HD + 1],
                bvb_bc.rearrange("p (h c) -> p h c", c=VW)[:, :, HD:HD + 1])
        # unnormalized attention outputs + normalized outputs, per pair
        outU = [opool.tile([P, T], BF16, tag="oU", name=f"outU{i}")
                for i in range(NPAIR)] if BUILD_STAGE >= 3 else None
        outT = [opool.tile([P, T], BF16, tag="oT", name=f"outT{i}")
                for i in range(NPAIR)] if BUILD_STAGE >= 4 else None

        # ---- Q/K projections: qT[o, t] = sum_k (WqT[k,o]).T @ xT[k, t] --
        def qk_proj(xdram, w_sb, bias_sb, dsts, nm):
            xt = [xpool.tile([P, T], BF16, tag="x", name=f"x{nm}{k}")
                  for k in range(KT)]
            for k in range(KT):
                nc.sync.dma_start(xt[k][:], xdram[k * P:(k + 1) * P, :])
            for m in range(NPAIR):
                for n in range(NSQ):
                    ps = psc.tile([P, 2 * SQ_BLK], F32, tag="sc")
                    for k in range(KT):
                        nc.tensor.matmul(
                            ps[:, 0:SQ_BLK],
                            w_sb[:, k, m * P:(m + 1) * P],
                            xt[k][:, n * SQ_BLK:(n + 1) * SQ_BLK],
                            start=(k == 0), stop=(k == KT - 1))
                    for X in range(2):
                        h = 2 * m + X
                        nc.vector.tensor_scalar_add(
                            dsts[h][0:HD, n * SQ_BLK:(n + 1) * SQ_BLK],
                            ps[X * HD:(X + 1) * HD, 0:SQ_BLK],
                            bias_sb[0:HD, h:h + 1])

        qk_proj(xq, wq_sb, bq_sb, qT, "q")
        qk_proj(xk, wk_sb, bk_sb, kT, "k")

        # ---- V projection: v[t, o] = (xT[k,t]).T @ WvT[k, o] (+ bv) ----
        xt = [xpool.tile([P, T], BF16, tag="x", name=f"xv{k}")
              for k in range(KT)]
        for k in range(KT):
            nc.sync.dma_start(xt[k][:], xv[k * P:(k + 1) * P, :])
        for tt in range(NSK):
            ps = pso.tile([P, SQ_BLK], F32, tag="o")
            for k in range(KT):
                nc.tensor.matmul(
                    ps[:, 0:LOCAL_E],
                    xt[k][:, tt * P:(tt + 1) * P],
                    wv_sb[:, k, :],
                    start=(k == 0), stop=(k == KT - 1))
            nc.vector.tensor_tensor(
                vbuf.rearrange("p s (h c) -> p s h c", c=VW)[:, tt, :, 0:HD],
                ps[:, 0:LOCAL_E].rearrange("p (h c) -> p h c", c=HD),
                bvb_bc.rearrange("p (h c) -> p h c", c=VW)[:, :, 0:HD],
                mybir.AluOpType.add)

        if DEBUG_DUMPS:
            nc.sync.dma_start(dbg["d_qT0"][:], qT[0][:])
            nc.sync.dma_start(dbg["d_kT0"][:], kT[0][:])
            nc.sync.dma_start(dbg["d_vbuf"][:],
                              vbuf.rearrange("p s c -> p (s c)"))
            nc.sync.dma_start(dbg["d_bvbbc"][:], bvb_bc[:])

        # ---- attention per head pair ----------------------------------
        for pr in range(NPAIR):
            for sq in range(NSQ):
                sqs = slice(sq * SQ_BLK, (sq + 1) * SQ_BLK)
                po = [pso.tile([P, SQ_BLK], F32, tag="o",
                               name=f"po{pr}_{sq}_{i}") for i in range(2)]
                if BUILD_STAGE < 2:
                    break
                for sk in range(NSK):
                    sks = slice(sk * P, (sk + 1) * P)
                    ps = psc.tile([P, 2 * SQ_BLK], F32, tag="sc")
                    # scoresT for both heads of the pair
                    for X in range(2):
                        h = 2 * pr + X
                        nc.tensor.matmul(
                            ps[:, X * SQ_BLK:(X + 1) * SQ_BLK],
                            kT[h][0:HD, sks], qT[h][0:HD, sqs],
                            start=True, stop=True)
                    ex = epool.tile([P, 2 * SQ_BLK], BF16, tag="e")
                    nc.scalar.activation(ex[:], ps[:], EXPF)
                    if DEBUG_DUMPS and pr == 0 and sq == 0 and sk == 0:
                        nc.sync.dma_start(dbg["d_ex0"][:], ex[:])
                    if BUILD_STAGE < 3:
                        exd = spool.tile([P, 2 * SQ_BLK], BF16, tag="exd",
                                         name=f"exd{pr}_{sq}_{sk}")
                        nc.vector.tensor_copy(exd[:], ex[:])
                        continue
                    for X in range(2):
                        h = 2 * pr + X
                        nc.tensor.matmul(
                            po[X][0:VW, :],
                            vbuf[:, sk, h * VW:(h + 1) * VW],
                            ex[:, X * SQ_BLK:(X + 1) * SQ_BLK],
                            start=(sk == 0), stop=(sk == NSK - 1))
                # evict + normalize: raw out -> outU (frees PSUM), then
                # 1/denom (approx, ~18 bits) -> f32r -> K=1 bcast matmul ->
                # bf16 -> outT = outU * bcast(1/den)
                if BUILD_STAGE < 3:
                    continue
                for X in range(2):
                    nc.vector.tensor_copy(
                        outU[pr][X * HD:(X + 1) * HD, sqs], po[X][0:HD, :])
                    if BUILD_STAGE < 4:
                        continue
                    dn = spool.tile([1, SQ_BLK], F32, tag="dn",
                                    name=f"dn{pr}_{sq}_{X}")
                    nc.vector.tensor_copy(dn[:], po[X][HD:VW, :])
                    rc = spool.tile([1, SQ_BLK], F32, tag="rc",
                                    name=f"rc{pr}_{sq}_{X}")
                    nc.vector.reciprocal_approx_fast(rc[:], dn[:])
                    if DEBUG_DUMPS and pr == 0 and sq == 0 and X == 0:
                        nc.sync.dma_start(dbg["d_rc0"][:], rc[:])
                    rcr = spool.tile([1, SQ_BLK], F32R, tag="rcr",
                                     name=f"rcr{pr}_{sq}_{X}")
                    nc.vector.tensor_copy(rcr[:], rc[:])
                    pb = pso.tile([P, SQ_BLK], F32, tag="o",
                                  name=f"pb{pr}_{sq}_{X}")
                    nc.tensor.matmul(pb[0:HD, :], onesr_sb[0:1, 0:HD],
                                     rcr[:], start=True, stop=True)
                    bc = spool.tile([P, SQ_BLK], BF16, tag="bc",
                                    name=f"bc{pr}_{sq}_{X}")
                    nc.vector.tensor_copy(bc[X * HD:(X + 1) * HD, :],
                                          pb[0:HD, :])
                    if DEBUG_DUMPS and pr == 0 and sq == 0 and X == 0:
                        nc.sync.dma_start(dbg["d_bc0"][:], bc[:])
                    nc.vector.tensor_tensor(
                        outT[pr][X * HD:(X + 1) * HD, sqs],
                        outU[pr][X * HD:(X + 1) * HD, sqs],
                        bc[X * HD:(X + 1) * HD, :], mybir.AluOpType.mult)

        if DEBUG_DUMPS and BUILD_STAGE >= 3:
            nc.sync.dma_start(dbg["d_outU0"][:], outU[0][:])
            if BUILD_STAGE >= 4:
                nc.sync.dma_start(dbg["d_outT0"][:], outT[0][:])

        # ---- output projection: finalT[e, t] = WoT.T @ outT ------------
        for m in range(E // P if BUILD_STAGE >= 5 else 0):
            for t in range(NSQ):
                ps = pso.tile([P, SQ_BLK], F32, tag="o")
                for kb in range(NPAIR):
                    nc.tensor.matmul(
                        ps[:],
                        wo_sb[:, kb, m * P:(m + 1) * P],
                        outT[kb][:, t * SQ_BLK:(t + 1) * SQ_BLK],
                        start=(kb == 0), stop=(kb == NPAIR - 1))
                fin = fpool.tile([P, SQ_BLK], F32, tag="f")
                nc.vector.tensor_copy(fin[:], ps[:])
                nc.sync.dma_start(
                    outp[m * P:(m + 1) * P, t * SQ_BLK:(t + 1) * SQ_BLK],
                    fin[:])

    nc.compile()
    return nc


_NC = None


def _get_nc():
    global _NC
    if _NC is None:
        _NC = _build_program()
    return _NC


def _bf(a):
    return np.ascontiguousarray(a.astype(NPBF16))


def _make_in_maps(inputs):
    q = np.asarray(inputs["query"], np.float32)
    k = np.asarray(inputs["key"], np.float32)
    v = np.asarray(inputs["value"], np.float32)
    Wq = np.asarray(inputs["Wq"], np.float32)
    Wk = np.asarray(inputs["Wk"], np.float32)
    Wv = np.asarray(inputs["Wv"], np.float32)
    Wo = np.asarray(inputs["Wo"], np.float32)
    bq = np.asarray(inputs["bq"], np.float32)
    bk = np.asarray(inputs["bk"], np.float32)
    bv = np.asarray(inputs["bv"], np.float32)
    scale = np.float32(HD ** -0.5)

    in_maps = []
    for c in range(NCORES):
        b = c // CORES_PER_BATCH
        h0 = (c % CORES_PER_BATCH) * HEADS_PER_CORE
        hsl = slice(h0 * HD, (h0 + HEADS_PER_CORE) * HD)
        bvh = bv[hsl].reshape(HEADS_PER_CORE, HD)
        bvb = np.concatenate(
            [bvh, np.ones((HEADS_PER_CORE, 1), np.float32)], axis=1).ravel()
        in_maps.append({
            "xq": _bf(q[:, b, :].T),
            "xk": _bf(k[:, b, :].T),
            "xv": _bf(v[:, b, :].T),
            "wqt": _bf((Wq[hsl, :] * scale).T),
            "wkt": _bf(Wk[hsl, :].T),
            "wvt": _bf(Wv[hsl, :].T),
            "wot": _bf(Wo[:, hsl].T),
            "bqh": np.ascontiguousarray(bq[hsl] * scale),
            "bkh": np.ascontiguousarray(bk[hsl]),
            "bvb": np.ascontiguousarray(bvb.astype(np.float32)),
            "onesc": np.ones(P, NPBF16),
            "onescr": np.ones(P, np.float32),
        })
    return in_maps


def run_sharded(inputs, trace=False):
    """Returns (full_output [S,B,E] f32, BassKernelResults)."""
    nc = _get_nc()
    in_maps = _make_in_maps(inputs)
    res = run_bass_kernel_spmd(nc, in_maps, core_ids=list(range(NCORES)),
                               trace=trace)
    bo = np.asarray(inputs["bo"], np.float32)
    final = np.zeros((S, B, E), np.float32)
    for c in range(NCORES):
        b = c // CORES_PER_BATCH
        final[:, b, :] += res.results[c]["outp"].T
    final += bo
    return final, res


def kernel(**inputs):
    out, _ = run_sharded(inputs, trace=False)
    return out


# revision 21
# speedup vs baseline: 1.0060x; 1.0060x over previous
"""Multi-head attention (S=2048, B=2, E=1024, H=16) on 8 Trainium2 cores.

Sharding: data-parallel over batch (4 cores per batch element) x tensor-parallel
over heads (4 heads per core), Megatron-style: Wq/Wk/Wv column-sharded,
Wo row-sharded, per-core partial outputs summed (+bo) on host.

Per-core device kernel, all activations in [feature, token] layout (all
transposes are free on the host):
  qT = WqT.T @ xT -> [256, 2048]; kT likewise; v = xT.T @ WvT -> [2048, 256]
  per head pair (heads 2p, 2p+1 row-packed in the PE array for scores):
    scoresT[sk, sq] = kT_h.T @ qT_h            (K=64, tile_position packed)
    expT = exp(scoresT)                        (ACT, PSUM->SBUF, bf16 out)
    out_h[(d|den), sq] += [v_h|1].T.T @ expT   (M=65; row 64 = softmax denom)
    evict raw out+den immediately (frees PSUM; denom rows batched)
  one reciprocal over all 16 denom rows; per (pair,sq) one dual K=1 broadcast
  matmul + one multiply normalizes: outT = outU * bcast(1/den)
  finalT_partial = WoT.T @ outT -> [1024, 2048]
Softmax needs no max-subtraction here: scores ~ N(0,1) (|max| < ~7), exp is
safe in fp32/bf16 range. exp@[v|1] then divide is exactly softmax@v.
Biases: bq,bk folded into projection evictions; bv added to v (rows of
softmax sum to 1, so attn@(v+bv) = attn@v + bv); bo added on host.
Matmuls run in bf16 (PE 1 cyc/row + fast weight load); PSUM accumulation and
softmax denominators stay fp32.
"""

import numpy as np
import ml_dtypes
from contextlib import ExitStack

import concourse.bass as bass
import concourse.mybir as mybir
from concourse import bacc
import concourse.tile as tile
from concourse.bass_utils import run_bass_kernel_spmd

S, B, E, H, HD = 2048, 2, 1024, 16, 64
P = 128
NCORES = 8
CORES_PER_BATCH = 4
HEADS_PER_CORE = H // CORES_PER_BATCH      # 4
LOCAL_E = HEADS_PER_CORE * HD              # 256
VW = HD + 1                                # 65: [v_h | ones]
T = S                                      # tokens per core (one batch elem)
KT = E // P                                # 8 contraction tiles for projections
NPAIR = HEADS_PER_CORE // 2                # 2 head pairs
SQ_BLK = 512
NSQ = T // SQ_BLK                          # 4
NSK = T // P                               # 16
F32 = mybir.dt.float32
F32R = mybir.dt.float32r
BF16 = mybir.dt.bfloat16
NPBF16 = ml_dtypes.bfloat16
EXPF = mybir.ActivationFunctionType.Exp


DEBUG_DUMPS = False
BUILD_STAGE = 5  # 1=proj 2=+scores/exp 3=+PV 4=+normalize 5=full


def _build_program():
    nc = bacc.Bacc("TRN2")

    xq = nc.dram_tensor("xq", [E, T], BF16, kind="ExternalInput")
    xk = nc.dram_tensor("xk", [E, T], BF16, kind="ExternalInput")
    xv = nc.dram_tensor("xv", [E, T], BF16, kind="ExternalInput")
    wqt = nc.dram_tensor("wqt", [E, LOCAL_E], BF16, kind="ExternalInput")
    wkt = nc.dram_tensor("wkt", [E, LOCAL_E], BF16, kind="ExternalInput")
    wvt = nc.dram_tensor("wvt", [E, LOCAL_E], BF16, kind="ExternalInput")
    wot = nc.dram_tensor("wot", [LOCAL_E, E], BF16, kind="ExternalInput")
    bqh = nc.dram_tensor("bqh", [LOCAL_E], F32, kind="ExternalInput")
    bkh = nc.dram_tensor("bkh", [LOCAL_E], F32, kind="ExternalInput")
    # per head: [bv_head (64), 1.0] -> 65 columns
    bvb = nc.dram_tensor("bvb", [HEADS_PER_CORE * VW], F32R,
                         kind="ExternalInput")
    onesc = nc.dram_tensor("onesc", [P], BF16, kind="ExternalInput")
    onescr = nc.dram_tensor("onescr", [P], F32R, kind="ExternalInput")
    outp = nc.dram_tensor("outp", [E, T], F32, kind="ExternalOutput")
    dbg = {}
    if DEBUG_DUMPS:
        for nm, shp, dt_ in [("d_qT0", [P, T], BF16), ("d_kT0", [P, T], BF16),
                             ("d_vbuf", [P, NSK * HEADS_PER_CORE * VW], BF16),
                             ("d_outU0", [P, T], BF16), ("d_outT0", [P, T], BF16),
                             ("d_ex0", [P, 2 * SQ_BLK], BF16),
                             ("d_rc0", [1, SQ_BLK], F32),
                             ("d_bc0", [P, SQ_BLK], BF16),
                             ("d_bvbbc", [P, HEADS_PER_CORE * VW], F32)]:
            dbg[nm] = nc.dram_tensor(nm, shp, dt_, kind="ExternalOutput")

    with ExitStack() as ctx:
        ctx.enter_context(nc.allow_low_precision(reason="bf16 matmul pipeline"))
        tc = ctx.enter_context(tile.TileContext(nc))
        xpool = ctx.enter_context(tc.tile_pool(name="xpool", bufs=16))
        wpool = ctx.enter_context(tc.tile_pool(name="wpool", bufs=1))
        qkpool = ctx.enter_context(tc.tile_pool(name="qkpool", bufs=8))
        vpool = ctx.enter_context(tc.tile_pool(name="vpool", bufs=1))
        opool = ctx.enter_context(tc.tile_pool(name="opool", bufs=4))
        epool = ctx.enter_context(tc.tile_pool(name="epool", bufs=4))
        fpool = ctx.enter_context(tc.tile_pool(name="fpool", bufs=4))
        spool = ctx.enter_context(tc.tile_pool(name="spool", bufs=4))
        cpool = ctx.enter_context(tc.tile_pool(name="cpool", bufs=1))
        # PSUM: psc 2x[128,1024] (scores + q/k proj) = 4 banks,
        #       pso 4x[128,512] (PV out / v-proj / bcast / o-proj) = 4 banks.
        psc = ctx.enter_context(tc.tile_pool(name="psc", bufs=2, space="PSUM"))
        pso = ctx.enter_context(tc.tile_pool(name="pso", bufs=4, space="PSUM"))

        # ---- constants -------------------------------------------------
        ones_sb = cpool.tile([1, P], BF16, tag="ones")
        nc.sync.dma_start(ones_sb[:], onesc[None, :])
        onesr_sb = cpool.tile([1, P], F32R, tag="onesr")
        nc.sync.dma_start(onesr_sb[:], onescr[None, :])
        bvb_sb = cpool.tile([1, HEADS_PER_CORE * VW], F32R, tag="bvbrow")
        nc.sync.dma_start(bvb_sb[:], bvb[None, :])
        bq_sb = cpool.tile([HD, HEADS_PER_CORE], F32, tag="bq")
        nc.sync.dma_start(bq_sb[:], bqh.rearrange("(h p) -> p h", p=HD))
        bk_sb = cpool.tile([HD, HEADS_PER_CORE], F32, tag="bk")
        nc.sync.dma_start(bk_sb[:], bkh.rearrange("(h p) -> p h", p=HD))
        # broadcast [bv_h | 1] over all 128 partitions via a K=1 outer product
        bvb_ps = pso.tile([P, SQ_BLK], F32, tag="o")
        nc.tensor.matmul(bvb_ps[:, 0:HEADS_PER_CORE * VW],
                         onesr_sb[0:1, :], bvb_sb[0:1, :],
                         start=True, stop=True)
        bvb_bc = cpool.tile([P, HEADS_PER_CORE * VW], F32, tag="bvbbc")
        nc.vector.tensor_copy(bvb_bc[:], bvb_ps[:, 0:HEADS_PER_CORE * VW])

        # ---- weights ---------------------------------------------------
        wq_sb = wpool.tile([P, KT, LOCAL_E], BF16, tag="wq")
        nc.sync.dma_start(wq_sb[:], wqt.rearrange("(k p) n -> p k n", p=P))
        wk_sb = wpool.tile([P, KT, LOCAL_E], BF16, tag="wk")
        nc.sync.dma_start(wk_sb[:], wkt.rearrange("(k p) n -> p k n", p=P))
        wv_sb = wpool.tile([P, KT, LOCAL_E], BF16, tag="wv")
        nc.sync.dma_start(wv_sb[:], wvt.rearrange("(k p) n -> p k n", p=P))
        wo_sb = wpool.tile([P, LOCAL_E // P, E], BF16, tag="wo")
        nc.sync.dma_start(wo_sb[:], wot.rearrange("(k p) n -> p k n", p=P))

        # ---- persistent activations -----------------------------------
        # per-head tiles (only rows 0:HD used) so every matmul lhsT/rhs
        # sits at base partition 0 -- no PE tile_position games
        qT = [qkpool.tile([P, T], BF16, tag="qk", name=f"qT{i}")
              for i in range(HEADS_PER_CORE)]
        kT = [qkpool.tile([P, T], BF16, tag="qk", name=f"kT{i}")
              for i in range(HEADS_PER_CORE)]
        # v buffer: per sk-tile, per head: [v_h (64 cols) | ones (1 col)]
        vbuf = vpool.tile([P, NSK, HEADS_PER_CORE * VW], BF16, tag="v")
        for _tt in range(NSK):
            nc.vector.tensor_copy(
                vbuf.rearrange("p s (h c) -> p s h c", c=VW)
                [:, _tt, :, HD:HD + 1],
                bvb_bc.rearrange("p (h c) -> p h c", c=VW)[:, :, HD:HD + 1])
        # unnormalized attention outputs + normalized outputs, per pair
        outU = [opool.tile([P, T], BF16, tag="oU", name=f"outU{i}")
                for i in range(NPAIR)] if BUILD_STAGE >= 3 else None
        outT = [opool.tile([P, T], BF16, tag="oT", name=f"outT{i}")
                for i in range(NPAIR)] if BUILD_STAGE >= 4 else None

        # ---- Q/K projections: qT[o, t] = sum_k (WqT[k,o]).T @ xT[k, t] --
        def qk_proj(xdram, w_sb, bias_sb, dsts, nm):
            xt = [xpool.tile([P, T], BF16, tag="x", name=f"x{nm}{k}")
                  for k in range(KT)]
            for k in range(KT):
                nc.sync.dma_start(xt[k][:], xdram[k * P:(k + 1) * P, :])
            for m in range(NPAIR):
                for n in range(NSQ):
                    ps = psc.tile([P, 2 * SQ_BLK], F32, tag="sc")
                    for k in range(KT):
                        nc.tensor.matmul(
                            ps[:, 0:SQ_BLK],
                            w_sb[:, k, m * P:(m + 1) * P],
                            xt[k][:, n * SQ_BLK:(n + 1) * SQ_BLK],
                            start=(k == 0), stop=(k == KT - 1))
                    for X in range(2):
                        h = 2 * m + X
                        nc.vector.tensor_scalar_add(
                            dsts[h][0:HD, n * SQ_BLK:(n + 1) * SQ_BLK],
                            ps[X * HD:(X + 1) * HD, 0:SQ_BLK],
                            bias_sb[0:HD, h:h + 1])

        qk_proj(xq, wq_sb, bq_sb, qT, "q")
        qk_proj(xk, wk_sb, bk_sb, kT, "k")

        # ---- V projection: v[t, o] = (xT[k,t]).T @ WvT[k, o] (+ bv) ----
        xt = [xpool.tile([P, T], BF16, tag="x", name=f"xv{k}")
              for k in range(KT)]
        for k in range(KT):
            nc.sync.dma_start(xt[k][:], xv[k * P:(k + 1) * P, :])
        for tt in range(NSK):
            ps = pso.tile([P, SQ_BLK], F32, tag="o")
            for k in range(KT):
                nc.tensor.matmul(
                    ps[:, 0:LOCAL_E],
                    xt[k][:, tt * P:(tt + 1) * P],
                    wv_sb[:, k, :],
                    start=(k == 0), stop=(k == KT - 1))
            nc.vector.tensor_tensor(
                vbuf.rearrange("p s (h c) -> p s h c", c=VW)[:, tt, :, 0:HD],
                ps[:, 0:LOCAL_E].rearrange("p (h c) -> p h c", c=HD),
                bvb_bc.rearrange("p (h c) -> p h c", c=VW)[:, :, 0:HD],
                mybir.AluOpType.add)

        if DEBUG_DUMPS:
            nc.sync.dma_start(dbg["d_qT0"][:], qT[0][:])
            nc.sync.dma_start(dbg["d_kT0"][:], kT[0][:])
            nc.sync.dma_start(dbg["d_vbuf"][:],
                              vbuf.rearrange("p s c -> p (s c)"))
            nc.sync.dma_start(dbg["d_bvbbc"][:], bvb_bc[:])

        # ---- attention per head pair ----------------------------------
        for pr in range(NPAIR):
            for sq in range(NSQ):
                sqs = slice(sq * SQ_BLK, (sq + 1) * SQ_BLK)
                po = [pso.tile([P, SQ_BLK], F32, tag="o",
                               name=f"po{pr}_{sq}_{i}") for i in range(2)]
                if BUILD_STAGE < 2:
                    break
                for sk in range(NSK):
                    sks = slice(sk * P, (sk + 1) * P)
                    ps = psc.tile([P, 2 * SQ_BLK], F32, tag="sc")
                    # scoresT for both heads of the pair
                    for X in range(2):
                        h = 2 * pr + X
                        nc.tensor.matmul(
                            ps[:, X * SQ_BLK:(X + 1) * SQ_BLK],
                            kT[h][0:HD, sks], qT[h][0:HD, sqs],
                            start=True, stop=True)
                    ex = epool.tile([P, 2 * SQ_BLK], BF16, tag="e")
                    nc.scalar.activation(ex[:], ps[:], EXPF)
                    if DEBUG_DUMPS and pr == 0 and sq == 0 and sk == 0:
                        nc.sync.dma_start(dbg["d_ex0"][:], ex[:])
                    if BUILD_STAGE < 3:
                        exd = spool.tile([P, 2 * SQ_BLK], BF16, tag="exd",
                                         name=f"exd{pr}_{sq}_{sk}")
                        nc.vector.tensor_copy(exd[:], ex[:])
                        continue
                    for X in range(2):
                        h = 2 * pr + X
                        nc.tensor.matmul(
                            po[X][0:VW, :],
                            vbuf[:, sk, h * VW:(h + 1) * VW],
                            ex[:, X * SQ_BLK:(X + 1) * SQ_BLK],
                            start=(sk == 0), stop=(sk == NSK - 1))
                # evict + normalize: raw out -> outU (frees PSUM), then
                # 1/denom (approx, ~18 bits) -> f32r -> K=1 bcast matmul ->
                # bf16 -> outT = outU * bcast(1/den)
                if BUILD_STAGE < 3:
                    continue
                for X in range(2):
                    nc.vector.tensor_copy(
                        outU[pr][X * HD:(X + 1) * HD, sqs], po[X][0:HD, :])
                    if BUILD_STAGE < 4:
                        continue
                    dn = spool.tile([1, SQ_BLK], F32, tag="dn",
                                    name=f"dn{pr}_{sq}_{X}")
                    nc.vector.tensor_copy(dn[:], po[X][HD:VW, :])
                    rc = spool.tile([1, SQ_BLK], F32, tag="rc",
                                    name=f"rc{pr}_{sq}_{X}")
                    nc.vector.reciprocal_approx_fast(rc[:], dn[:])
                    if DEBUG_DUMPS and pr == 0 and sq == 0 and X == 0:
                        nc.sync.dma_start(dbg["d_rc0"][:], rc[:])
                    rcr = spool.tile([1, SQ_BLK], F32R, tag="rcr",
                                     name=f"rcr{pr}_{sq}_{X}")
                    nc.vector.tensor_copy(rcr[:], rc[:])
                    pb = pso.tile([P, SQ_BLK], F32, tag="o",
                                  name=f"pb{pr}_{sq}_{X}")
                    nc.tensor.matmul(pb[0:HD, :], onesr_sb[0:1, 0:HD],
                                     rcr[:], start=True, stop=True)
                    bc = spool.tile([P, SQ_BLK], BF16, tag="bc",
                                    name=f"bc{pr}_{sq}_{X}")
                    nc.vector.tensor_copy(bc[X * HD:(X + 1) * HD, :],
                                          pb[0:HD, :])
                    if DEBUG_DUMPS and pr == 0 and sq == 0 and X == 0:
                        nc.sync.dma_start(dbg["d_bc0"][:], bc[:])
                    nc.vector.tensor_tensor(
                        outT[pr][X * HD:(X + 1) * HD, sqs],
                        outU[pr][X * HD:(X + 1) * HD, sqs],
                        bc[X * HD:(X + 1) * HD, :], mybir.AluOpType.mult)

        if DEBUG_DUMPS and BUILD_STAGE >= 3:
            nc.sync.dma_start(dbg["d_outU0"][:], outU[0][:])
            if BUILD_STAGE >= 4:
                nc.sync.dma_start(dbg["d_outT0"][:], outT[0][:])

        # ---- output projection: finalT[e, t] = WoT.T @ outT ------------
        for m in range(E // P if BUILD_STAGE >= 5 else 0):
            for t in range(NSQ):
                ps = pso.tile([P, SQ_BLK], F32, tag="o")
                for kb in range(NPAIR):
                    nc.tensor.matmul(
                        ps[:],
                        wo_sb[:, kb, m * P:(m + 1) * P],
                        outT[kb][:, t * SQ_BLK:(t + 1) * SQ_BLK],
                        start=(kb == 0), stop=(kb == NPAIR - 1))
                fin = fpool.tile([P, SQ_BLK], F32, tag="f")
                nc.vector.tensor_copy(fin[:], ps[:])
                nc.sync.dma_start(
                    outp[m * P:(m + 1) * P, t * SQ_BLK:(t + 1) * SQ_BLK],
                    fin[:])

    nc.compile()
    return nc


_NC = None


def _get_nc():
    global _NC
    if _NC is None:
        _NC = _build_program()
    return _NC


def _bf(a):
    return np.ascontiguousarray(a.astype(NPBF16))


def _make_in_maps(inputs):
    q = np.asarray(inputs["query"], np.float32)
    k = np.asarray(inputs["key"], np.float32)
    v = np.asarray(inputs["value"], np.float32)
    Wq = np.asarray(inputs["Wq"], np.float32)
    Wk = np.asarray(inputs["Wk"], np.float32)
    Wv = np.asarray(inputs["Wv"], np.float32)
    Wo = np.asarray(inputs["Wo"], np.float32)
    bq = np.asarray(inputs["bq"], np.float32)
    bk = np.asarray(inputs["bk"], np.float32)
    bv = np.asarray(inputs["bv"], np.float32)
    scale = np.float32(HD ** -0.5)

    in_maps = []
    for c in range(NCORES):
        b = c // CORES_PER_BATCH
        h0 = (c % CORES_PER_BATCH) * HEADS_PER_CORE
        hsl = slice(h0 * HD, (h0 + HEADS_PER_CORE) * HD)
        bvh = bv[hsl].reshape(HEADS_PER_CORE, HD)
        bvb = np.concatenate(
            [bvh, np.ones((HEADS_PER_CORE, 1), np.float32)], axis=1).ravel()
        in_maps.append({
            "xq": _bf(q[:, b, :].T),
            "xk": _bf(k[:, b, :].T),
            "xv": _bf(v[:, b, :].T),
            "wqt": _bf((Wq[hsl, :] * scale).T),
            "wkt": _bf(Wk[hsl, :].T),
            "wvt": _bf(Wv[hsl, :].T),
            "wot": _bf(Wo[:, hsl].T),
            "bqh": np.ascontiguousarray(bq[hsl] * scale),
            "bkh": np.ascontiguousarray(bk[hsl]),
            "bvb": np.ascontiguousarray(bvb.astype(np.float32)),
            "onesc": np.ones(P, NPBF16),
            "onescr": np.ones(P, np.float32),
        })
    return in_maps


def run_sharded(inputs, trace=False):
    """Returns (full_output [S,B,E] f32, BassKernelResults)."""
    nc = _get_nc()
    in_maps = _make_in_maps(inputs)
    res = run_bass_kernel_spmd(nc, in_maps, core_ids=list(range(NCORES)),
                               trace=trace)
    bo = np.asarray(inputs["bo"], np.float32)
    final = np.zeros((S, B, E), np.float32)
    for c in range(NCORES):
        b = c // CORES_PER_BATCH
        final[:, b, :] += res.results[c]["outp"].T
    final += bo
    return final, res


def kernel(**inputs):
    out, _ = run_sharded(inputs, trace=False)
    return out
